# revision 3
# baseline (speedup 1.0000x reference)
"""Multi-head attention (B=2, S=2048, D=1024, H=16, d_head=64) on 8 TRN2 cores.

Sharding: 2-way data parallel over batch x 4-way tensor parallel over heads.
Core c: batch g = c//4, heads [4r, 4r+4) with r = c%4.

v2: single fused pipeline. The scalar engine's exp stream is the critical
resource (~1.4us per [128,1024] tile, 128 tiles); everything else is
scheduled to hide beneath it:
  - K proj + Q proj(q4=0) run first (dt-outer, 8 PSUM banks), so the first
    exp issues at ~19us instead of ~85us.
  - V proj, Q proj(q4>=1), and Wout chunks accumulate in a 2-bank aux PSUM
    pool and are interleaved between attention iterations in PE program
    order, filling the PE slack under the exp stream.
  - Softmax normalization avoids the DRAM round-trip: the denominator row
    (PSUM partition 64) is reciprocal'd in place (DVE, bf16 out), broadcast
    across 64 partitions with a K=1 ones matmul (PE), then fused into the
    DVE multiply that writes the bf16 head staging tile.
  - AllGather is split per (q4, head-pair): 8 collectives of [128,512]
    instead of 4 of [256,512], so the last collective on the critical tail
    carries half the data; Wout(q4) is interleaved into later sweeps.
"""

import os
import sys

import numpy as np

for _p in ("/opt/trn_rl_repo",):
    if _p not in sys.path and os.path.isdir(_p):
        sys.path.append(_p)

import ml_dtypes

import concourse.bacc as bacc
import concourse.mybir as mybir
from concourse.bass_utils import run_bass_kernel_spmd
from concourse.tile import TileContext

P = 128
B, S, DM = 2, 2048, 1024
NH_TOT, EH = 16, 64
NCORES = 8
GROUPS = 2
NH = 4  # heads per core
EHC = NH * EH  # 256
NDT = DM // P  # 8
NKT = S // P  # 16
QC = 512
NQC = S // QC  # 4
VW = EH + 1  # V width incl. ones column

BF = mybir.dt.bfloat16
F32 = mybir.dt.float32

USE_APPROX_RCP = os.environ.get("USE_APPROX_RCP", "1") == "1"

_cached_nc = None


def voff(kt, h):
    return (kt * NH + h) * VW


def build_nc():
    nc = bacc.Bacc("TRN2", target_bir_lowering=False, debug=False, num_devices=NCORES)

    xqt = nc.declare_dram_parameter("xqt", [DM, S], BF, isOutput=False)
    xkt = nc.declare_dram_parameter("xkt", [DM, S], BF, isOutput=False)
    xvt = nc.declare_dram_parameter("xvt", [DM, S], BF, isOutput=False)
    wqt = nc.declare_dram_parameter("wqt", [DM, EHC], BF, isOutput=False)
    wkt = nc.declare_dram_parameter("wkt", [DM, EHC], BF, isOutput=False)
    wvt = nc.declare_dram_parameter("wvt", [DM, EHC], BF, isOutput=False)
    wot = nc.declare_dram_parameter("wot", [DM, EHC], BF, isOutput=False)
    outt = nc.declare_dram_parameter("outt", [EHC, S], F32, isOutput=True)

    with TileContext(nc) as tc:
        with (
            tc.tile_pool(name="persist", bufs=1) as persist,
            tc.tile_pool(name="dram", bufs=1, space="DRAM") as dram,
        ):
            # --- persistent SBUF ---
            wq_sb = persist.tile([P, NDT, EHC], BF)
            wk_sb = persist.tile([P, NDT, EHC], BF)
            wv_sb = persist.tile([P, NDT, EHC], BF)
            wo_sb = persist.tile([P, NDT, EHC], BF)
            # weight DMAs on the gpsimd queue; K/Q first (they gate the ramp)
            for wsb, wpar in ((wk_sb, wkt), (wq_sb, wqt), (wv_sb, wvt), (wo_sb, wot)):
                nc.gpsimd.dma_start(wsb[:], wpar.rearrange("(dt p) e -> p dt e", p=P))

            qt_sb = [persist.tile([P, S], BF, name=f"qt{et}") for et in range(2)]
            kt_sb = [persist.tile([P, S], BF, name=f"kt{et}") for et in range(2)]
            v_sb = persist.tile([P, NKT * NH * VW + P - VW], BF)
            nc.gpsimd.memset(v_sb[:], 1.0)  # ones columns; V data overwrites 0:64
            ones_sb = persist.tile([P, EH], BF)
            nc.gpsimd.memset(ones_sb[:], 1.0)
            xv_sb = persist.tile([P, NDT, S], BF)

            # Collectives: ~12-35us per op on one serial CC stream. The
            # stream is trigger-gated early and serialization-gated late, so
            # q4 0-1 use whole-quarter ops (fewer ops up front) while q4 2-3
            # are ep-split (the congested end segment drains in half-size
            # steps that start as soon as each head-pair finishes).
            hloc = [dram.tile([EHC, QC], BF, name=f"hloc{q4}") for q4 in range(2)]
            hgat = [dram.tile([4 * EHC, QC], BF, name=f"hgat{q4}") for q4 in range(2)]
            hlocs = {
                q4: [dram.tile([P, QC], BF, name=f"hlocs{q4}_{ep}") for ep in range(2)]
                for q4 in (2, 3)
            }
            hgats = {
                q4: [
                    dram.tile([4 * P, QC], BF, name=f"hgats{q4}_{ep}")
                    for ep in range(2)
                ]
                for q4 in (2, 3)
            }

            def _ag(src, dst):
                nc.gpsimd.collective_compute(
                    "AllGather",
                    mybir.AluOpType.bypass,
                    replica_groups=[[0, 1, 2, 3], [4, 5, 6, 7]],
                    ins=[src.opt()],
                    outs=[dst.opt()],
                )

            def emit_allgather(q4):
                _ag(hloc[q4], hgat[q4])

            def emit_allgather_s(q4, ep):
                _ag(hlocs[q4][ep], hgats[q4][ep])

            with (
                tc.tile_pool(name="xin", bufs=4) as xin,
                tc.tile_pool(name="xqp", bufs=2) as xqp,
            ):
                xqsrc = xqt.rearrange("(dt p) s -> p dt s", p=P)
                xvsrc = xvt.rearrange("(dt p) s -> p dt s", p=P)
                xq_t = {}

                def emit_xq_dma(q4):
                    xq_t[q4] = xqp.tile([P, NDT, QC], BF, name="xq", tag="xq")
                    nc.sync.dma_start(
                        xq_t[q4][:], xqsrc[:, :, q4 * QC : (q4 + 1) * QC]
                    )

                # --- K proj (dt-outer, 8 PSUM banks, own scope) ---
                with tc.tile_pool(name="projk", bufs=1, space="PSUM") as projk:
                    kps = [
                        [
                            projk.tile([P, QC], F32, name=f"kp{et}_{qc}")
                            for qc in range(NQC)
                        ]
                        for et in range(2)
                    ]
                    for dt in range(NDT):
                        xt = xin.tile([P, S], BF, name="xt", tag="xt")
                        nc.sync.dma_start(xt[:], xkt[dt * P : (dt + 1) * P, :])
                        for et in range(2):
                            for qc in range(NQC):
                                nc.tensor.matmul(
                                    kps[et][qc][:],
                                    wk_sb[:, dt, et * P : (et + 1) * P],
                                    xt[:, qc * QC : (qc + 1) * QC],
                                    start=(dt == 0),
                                    stop=(dt == NDT - 1),
                                )
                    # xq0 + xv slices land behind xk on the sync queue
                    emit_xq_dma(0)
                    for tt in range(NKT):
                        nc.sync.dma_start(
                            xv_sb[:, :, tt * P : (tt + 1) * P],
                            xvsrc[:, :, tt * P : (tt + 1) * P],
                        )
                    for et in range(2):
                        for qc in range(NQC):
                            nc.vector.tensor_copy(
                                kt_sb[et][:, qc * QC : (qc + 1) * QC], kps[et][qc][:]
                            )

                # --- Q proj for q4=0 (2 banks, own scope) ---
                with tc.tile_pool(name="projq", bufs=1, space="PSUM") as projq:
                    qps = [projq.tile([P, QC], F32, name=f"qp{et}") for et in range(2)]
                    for dt in range(NDT):
                        for et in range(2):
                            nc.tensor.matmul(
                                qps[et][:],
                                wq_sb[:, dt, et * P : (et + 1) * P],
                                xq_t[0][:, dt, :],
                                start=(dt == 0),
                                stop=(dt == NDT - 1),
                            )
                    for et in range(2):
                        nc.vector.tensor_copy(qt_sb[et][:, 0:QC], qps[et][:])

                # --- fused attention + aux pipeline ---
                with (
                    tc.tile_pool(name="scorep", bufs=2, space="PSUM") as scorep,
                    tc.tile_pool(name="pvp", bufs=1, space="PSUM") as pvp,
                    tc.tile_pool(name="auxp", bufs=2, space="PSUM") as auxp,
                    tc.tile_pool(name="expp", bufs=4) as expp,
                    tc.tile_pool(name="pvdp", bufs=2) as pvdp,
                    tc.tile_pool(name="rcpp", bufs=2) as rcpp,
                    tc.tile_pool(name="stgp", bufs=4) as stgp,
                    tc.tile_pool(name="outsp", bufs=2) as outsp,
                    tc.tile_pool(name="hallp", bufs=2) as hallp,
                ):
                    hall_t = {}  # q4 -> [8 tiles]

                    def emit_hall(q4, dts):
                        if q4 not in hall_t:
                            hall_t[q4] = [None] * NDT
                        for dt in dts:
                            t = hallp.tile([P, QC], BF, name=f"hl{dt}", tag=f"hl{dt}")
                            hall_t[q4][dt] = t
                            if q4 <= 1:
                                srcv = hgat[q4][
                                    (dt // 2) * EHC
                                    + (dt % 2) * P : (dt // 2) * EHC
                                    + (dt % 2) * P
                                    + P,
                                    :,
                                ]
                            else:
                                srcv = hgats[q4][dt % 2][
                                    (dt // 2) * P : (dt // 2) * P + P, :
                                ]
                            nc.sync.dma_start(t[:], srcv)

                    # --- aux chunk emitters; each returns a list of steps ---
                    def vproj_chunk(tt):
                        def go():
                            psv = auxp.tile([P, QC], F32, name="aux", tag="aux")
                            for dt in range(NDT):
                                nc.tensor.matmul(
                                    psv[:, 0:EHC],
                                    xv_sb[:, dt, tt * P : (tt + 1) * P],
                                    wv_sb[:, dt, :],
                                    start=(dt == 0),
                                    stop=(dt == NDT - 1),
                                    skip_group_check=True,
                                )
                            nc.vector.tensor_copy(
                                v_sb[:, tt * NH * VW : (tt + 1) * NH * VW].rearrange(
                                    "p (h w) -> p h w", w=VW
                                )[:, :, 0:EH],
                                psv[:, 0:EHC].rearrange("p (h e) -> p h e", e=EH),
                            )

                        return [go]

                    def qproj_chunk(q4, et):
                        state = {}

                        def part(dts, last):
                            def go():
                                if "ps" not in state:
                                    state["ps"] = auxp.tile(
                                        [P, QC], F32, name="aux", tag="aux"
                                    )
                                ps = state["ps"]
                                for dt in dts:
                                    nc.tensor.matmul(
                                        ps[:],
                                        wq_sb[:, dt, et * P : (et + 1) * P],
                                        xq_t[q4][:, dt, :],
                                        start=(dt == 0),
                                        stop=(dt == NDT - 1),
                                        skip_group_check=True,
                                    )
                                if last:
                                    nc.vector.tensor_copy(
                                        qt_sb[et][:, q4 * QC : (q4 + 1) * QC], ps[:]
                                    )

                            return go

                        return [part(range(0, 4), False), part(range(4, NDT), True)]

                    def wout_chunk(q4, ot):
                        state = {}

                        def part(dts, last):
                            def go():
                                if "ps" not in state:
                                    state["ps"] = auxp.tile(
                                        [P, QC], F32, name="aux", tag="aux"
                                    )
                                ps = state["ps"]
                                for dt in dts:
                                    nc.tensor.matmul(
                                        ps[:],
                                        wo_sb[:, dt, ot * P : (ot + 1) * P],
                                        hall_t[q4][dt][:],
                                        start=(dt == 0),
                                        stop=(dt == NDT - 1),
                                        skip_group_check=True,
                                    )
                                if last:
                                    ob = outsp.tile([P, QC], F32, name="ob", tag="ob")
                                    nc.vector.tensor_copy(ob[:], ps[:])
                                    nc.gpsimd.dma_start(
                                        outt[
                                            ot * P : (ot + 1) * P,
                                            q4 * QC : (q4 + 1) * QC,
                                        ],
                                        ob[:],
                                    )

                            return go

                        return [part(range(0, 4), False), part(range(4, NDT), True)]

                    # Sweep-end normalize, split in two so the PE never
                    # stalls: flush_a (kt==1 of the next sweep) drains the PV
                    # accumulator to SBUF with one DVE copy -- the only op the
                    # next sweep's first PV matmul waits on -- then computes
                    # the reciprocal off-critical. flush_b (kt==6) broadcasts
                    # it via a K=1 ones matmul, multiplies, stages to DRAM and
                    # triggers the collective.
                    def flush_a(pvt):
                        pvd = pvdp.tile([P, 2 * QC], F32, name="pvd", tag="pvd")
                        nc.vector.tensor_copy(pvd[0 : EH + 1, :], pvt[0 : EH + 1, :])
                        rcp = rcpp.tile([P, 2 * QC], F32, name="rcp", tag="rcpf")
                        if USE_APPROX_RCP:
                            # The custom DVE op mislowers at base_partition>0:
                            # run it over rows 0:65 (base 0; same duration --
                            # cost scales with the free dim). Rows 0:64 hold
                            # head values whose reciprocals are junk; only row
                            # 64 (the denominator) is ever read downstream.
                            nc.vector.reciprocal_approx_fast(
                                rcp[0 : EH + 1, :], pvd[0 : EH + 1, :]
                            )
                        else:
                            nc.vector.reciprocal(
                                rcp[EH : EH + 1, :], pvd[EH : EH + 1, :]
                            )
                        rcpb = rcpp.tile([P, 2 * QC], BF, name="rcpb", tag="rcpb")
                        nc.vector.tensor_copy(
                            rcpb[EH : EH + 1, :], rcp[EH : EH + 1, :]
                        )
                        return pvd, rcpb

                    def flush_b(q4, ep, fstate):
                        pvd, rcpb = fstate
                        bc = [None, None]
                        for lh in range(2):
                            bc[lh] = auxp.tile([P, QC], F32, name="bc", tag="aux")
                            nc.tensor.matmul(
                                bc[lh][0:EH, :],
                                ones_sb[EH : EH + 1, 0:EH],
                                rcpb[EH : EH + 1, lh * QC : (lh + 1) * QC],
                                start=True,
                                stop=True,
                            )
                        for lh in range(2):
                            stg = stgp.tile([P, QC], BF, name="stg", tag="stg")
                            nc.vector.tensor_mul(
                                stg[0:EH, :],
                                pvd[0:EH, lh * QC : (lh + 1) * QC],
                                bc[lh][0:EH, :],
                            )
                            if q4 <= 1:
                                dst = hloc[q4][
                                    ep * P + lh * EH : ep * P + (lh + 1) * EH, :
                                ]
                            else:
                                dst = hlocs[q4][ep][lh * EH : (lh + 1) * EH, :]
                            nc.gpsimd.dma_start(dst, stg[0:EH, :])
                        if q4 <= 1:
                            if ep == 1:
                                emit_allgather(q4)
                        else:
                            emit_allgather_s(q4, ep)

                    # per-sweep aux step schedule: {kt: [steps]}
                    def sweep_steps(s):
                        steps = {}

                        def put2(kts, chunk):
                            for kt, st in zip(kts, chunk):
                                steps.setdefault(kt, []).append(st)

                        if s == 0:
                            for tt in range(NKT):
                                steps.setdefault(tt, []).extend(vproj_chunk(tt))
                        elif s in (1, 2, 3):
                            q4n = s  # Q proj for q4 = s (needed by sweep 2*s)
                            put2((2, 4), qproj_chunk(q4n, 0))
                            put2((8, 10), qproj_chunk(q4n, 1))
                        elif s == 6:
                            put2((2, 4), wout_chunk(0, 0))
                            put2((8, 10), wout_chunk(0, 1))
                        elif s == 7:
                            put2((8, 10), wout_chunk(1, 0))
                            put2((12, 14), wout_chunk(1, 1))
                        return steps

                    pending = None  # (q4, ep, pvt) awaiting normalize
                    for q4 in range(NQC):
                        q0 = q4 * QC
                        for ep in range(2):
                            s = q4 * 2 + ep
                            steps = sweep_steps(s)
                            if s <= 2:
                                emit_xq_dma(s + 1)
                            pvt = pvp.tile([P, 2 * QC], F32, name="pv", tag="pv")
                            exring = [None] * NKT
                            for kt in range(NKT + 2):
                                if kt < NKT:
                                    exq = expp.tile(
                                        [P, 1024], BF, name="exq", tag="exq"
                                    )
                                    exring[kt] = exq
                                    s_t = scorep.tile(
                                        [P, 1024], F32, name="sq", tag="sq"
                                    )
                                    for lh in range(2):
                                        po = lh * EH
                                        nc.tensor.matmul(
                                            s_t[:, lh * QC : (lh + 1) * QC],
                                            kt_sb[ep][
                                                po : po + EH, kt * P : (kt + 1) * P
                                            ],
                                            qt_sb[ep][po : po + EH, q0 : q0 + QC],
                                            start=True,
                                            stop=True,
                                        )
                                    nc.scalar.activation(
                                        exq[:],
                                        s_t[:],
                                        mybir.ActivationFunctionType.Exp,
                                        scale=float(1.0 / np.sqrt(EH)),
                                    )
                                if kt == 1 and pending is not None:
                                    fstate = flush_a(pending[2])
                                if kt == 6 and pending is not None:
                                    flush_b(pending[0], pending[1], fstate)
                                    pending = None
                                if kt >= 2:
                                    pkt = kt - 2
                                    for lh in range(2):
                                        h = 2 * ep + lh
                                        nc.tensor.matmul(
                                            pvt[:, lh * QC : (lh + 1) * QC],
                                            v_sb[:, voff(pkt, h) : voff(pkt, h) + P],
                                            exring[pkt][:, lh * QC : (lh + 1) * QC],
                                            start=(pkt == 0),
                                            stop=(pkt == NKT - 1),
                                            skip_group_check=True,
                                        )
                                # hall loads: only after the producing AG was emitted
                                if kt == 0 and s == 5:
                                    emit_hall(0, range(NDT))
                                if kt == 12 and s == 6:
                                    emit_hall(1, range(NDT))
                                for st in steps.get(kt, ()):
                                    st()
                            pending = (q4, ep, pvt)

                    # --- tail: flush (3,1), Wout(2) under the last AGs,
                    # then Wout(3) with even (AG(3,0)) / odd (AG(3,1)) halves
                    fstate = flush_a(pending[2])
                    flush_b(pending[0], pending[1], fstate)
                    emit_hall(2, range(NDT))
                    emit_hall(3, (0, 2, 4, 6))
                    for ot in range(2):
                        for st in wout_chunk(2, ot):
                            st()
                    emit_hall(3, (1, 3, 5, 7))
                    for ot in range(2):
                        ps = auxp.tile([P, QC], F32, name="aux", tag="aux")
                        for i, dt in enumerate((0, 2, 4, 6, 1, 3, 5, 7)):
                            nc.tensor.matmul(
                                ps[:],
                                wo_sb[:, dt, ot * P : (ot + 1) * P],
                                hall_t[3][dt][:],
                                start=(i == 0),
                                stop=(i == NDT - 1),
                                skip_group_check=True,
                            )
                        ob = outsp.tile([P, QC], F32, name="ob", tag="ob")
                        nc.vector.tensor_copy(ob[:], ps[:])
                        nc.gpsimd.dma_start(
                            outt[ot * P : (ot + 1) * P, 3 * QC : 4 * QC], ob[:]
                        )

    nc.compile()
    return nc


def _prep_inputs(x_query, x_key, x_value, Wq, Wk, Wv, Wout):
    bf = ml_dtypes.bfloat16
    xt = {}
    for g in range(GROUPS):
        xt[g] = tuple(
            np.ascontiguousarray(np.asarray(x[g], dtype=np.float32).T).astype(bf)
            for x in (x_query, x_key, x_value)
        )
    in_maps = []
    for c in range(NCORES):
        g, r = c // 4, c % 4
        hs = slice(NH * r, NH * (r + 1))
        wq_c = np.ascontiguousarray(
            np.asarray(Wq[hs], dtype=np.float32).reshape(EHC, DM).T
        ).astype(bf)
        wk_c = np.ascontiguousarray(
            np.asarray(Wk[hs], dtype=np.float32).reshape(EHC, DM).T
        ).astype(bf)
        wv_c = np.ascontiguousarray(
            np.asarray(Wv[hs], dtype=np.float32).reshape(EHC, DM).T
        ).astype(bf)
        wo_c = np.ascontiguousarray(
            np.asarray(Wout[EHC * r : EHC * (r + 1), :], dtype=np.float32).T
        ).astype(bf)
        in_maps.append(
            {
                "xqt": xt[g][0],
                "xkt": xt[g][1],
                "xvt": xt[g][2],
                "wqt": wq_c,
                "wkt": wk_c,
                "wvt": wv_c,
                "wot": wo_c,
            }
        )
    return in_maps


def kernel(x_query, x_key, x_value, Wq, Wk, Wv, Wout, _trace=False):
    global _cached_nc
    if _cached_nc is None:
        _cached_nc = build_nc()
    nc = _cached_nc

    in_maps = _prep_inputs(x_query, x_key, x_value, Wq, Wk, Wv, Wout)
    res = run_bass_kernel_spmd(nc, in_maps, list(range(NCORES)), trace=_trace)
    kernel.last_result = res

    out = np.empty((B, S, DM), dtype=np.float32)
    for c in range(NCORES):
        g, r = c // 4, c % 4
        out[g, :, EHC * r : EHC * (r + 1)] = res.results[c]["outt"].T
    return out
